# revision 2
# baseline (speedup 1.0000x reference)
"""LoRA layer kernel for Trainium2 (Bass/Tile), data-parallel over 8 NeuronCores.

Math:  out = (x @ B) @ A * (32/16)   with x [4,2048,4096], B [4096,16], A [16,4096].

Design (HBM-bound: ~8 MB in + ~8 MB out per core at f16; floor ~42-47 us):
  - Flatten tokens (4*2048=8192), shard 1024 tokens per core (data parallel).
  - x pre-tiled PARTITION-MAJOR on host as [ntb, 128, NB, tb] f16 so loads are
    fully-contiguous fat descriptors per partition.
  - THREE DMA rings so load and store streams never serialize behind each
    other (SDMA round-robins rings at packet granularity):
      loads  -> nc.sync   (qSPDynamicHW)
      stores -> nc.scalar (qActDynamicHW) for odd token-subtiles,
                nc.gpsimd (SWDGE)         for even token-subtiles
  - ALL x blocks fully SBUF-buffered (bufs=ntb): every load is dispatched
    up-front with zero buffer-reuse waits; the DMA queue never starves.
  - Block 0 loaded in quarters so mm1 starts after ~512 KB instead of 2 MB;
    later blocks are single 2 MB DMAs (fat descriptors, fewer dispatches).
  - B halved vs old layout ([P, NB, R], no zero padding) -> faster first load
    (gates every mm1 LDWEIGHTS).
  - A loaded compact [16, 4096] 4x from DRAM straight into the 4 row-group
    positions (32j+r) mm2 needs — no on-chip replication copies on DVE.
  - mm1: 4-way column-group packed fp16 matmuls; chunk 4k+g accumulates
    into PSUM partitions [32g, 32g+16).
  - The 4 col-group partials are folded with a DVE add chain (one PSUM
    operand per op) straight into mm2's row-group weight layout.
  - mm2: fp16, row-group packed, SUBTILE-OUTER loop order so each output
    subtile's 8 chunks finish consecutively and its store dispatches early
    (overlapping the store stream with the tail of the load stream).
  - PSUM->SBUF output copies: even subtile -> DVE, odd -> ACT. The ACT-copied
    subtiles are stored from ACT itself (self-gated dispatch, no cross-engine
    stall); DVE-copied subtiles are stored from the otherwise-idle gpsimd.
"""

import os
import numpy as np

IN = 4096
OUT = 4096
R = 16
N_CORES = 8
SCALE = 32.0 / 16.0
P = 128
NB = IN // P  # 32 contraction chunks


def _install_profile_hook():
    """Best-effort: register the axon NTFF profiling hook that this image's
    `antenv` package is missing, so run_bass_kernel_spmd(trace=True) can
    return exec_time_ns. Harmless no-op when anything is unavailable."""
    try:
        import sys
        import types

        if "antenv.axon_hooks" in sys.modules:
            return
        try:
            import antenv  # noqa: F401
        except ImportError:
            return
        mod = types.ModuleType("antenv.axon_hooks")
        mod._hook = None

        def set_axon_ntff_profile_hook(h):
            mod._hook = h

        def get_axon_ntff_profile_hook():
            return mod._hook

        mod.set_axon_ntff_profile_hook = set_axon_ntff_profile_hook
        mod.get_axon_ntff_profile_hook = get_axon_ntff_profile_hook
        sys.modules["antenv.axon_hooks"] = mod
        import antenv as _antenv

        _antenv.axon_hooks = mod

        so_path = "/opt/axon/libaxon_pjrt.so"
        if os.path.exists(so_path):
            try:
                from trn_agent_boot.trn_boot import _ntff_profile_via_ctypes

                hook = _ntff_profile_via_ctypes(so_path)
                if hook is not None:
                    mod._hook = hook
            except Exception:
                pass
    except Exception:
        pass


_install_profile_hook()

_NC_CACHE = {}


def build_nc(tok, tb=256):
    """Build + compile the per-core Bass program for `tok` tokens/core."""
    key = (tok, tb)
    if key in _NC_CACHE:
        return _NC_CACHE[key]

    import concourse.bacc as bacc
    import concourse.tile as tile
    from concourse import mybir

    f32 = mybir.dt.float32
    f16 = mybir.dt.float16
    tb = min(tb, tok)
    assert tok % tb == 0 and tb % P == 0
    ntb = tok // tb
    nst = tb // P  # token subtiles per block

    nc = bacc.Bacc("TRN2", target_bir_lowering=False, debug=False)
    xT = nc.dram_tensor("xT", [ntb, P, NB, tb], f16, kind="ExternalInput").ap()
    Bt = nc.dram_tensor("Bt", [P, NB, R], f16, kind="ExternalInput").ap()
    Af = nc.dram_tensor("Af", [R, OUT], f16, kind="ExternalInput").ap()
    out = nc.dram_tensor("out", [tok, OUT], f16, kind="ExternalOutput").ap()

    with tile.TileContext(nc) as tc:
        with (
            tc.tile_pool(name="const", bufs=1) as const_pool,
            tc.tile_pool(name="xin", bufs=ntb) as x_pool,
            tc.tile_pool(name="xbt", bufs=2) as xbt_pool,
            tc.tile_pool(name="ps1", bufs=2, space="PSUM") as ps1,
            tc.tile_pool(name="ps2", bufs=6, space="PSUM") as ps2,
            tc.tile_pool(name="osb", bufs=min(4, ntb)) as out_pool,
        ):
            xT_sbs = [
                x_pool.tile([P, NB, tb], f16, name=f"x{i}", tag="x")
                for i in range(ntb)
            ]
            # tiny B first: it gates every mm1 LDWEIGHTS
            B_sb = const_pool.tile([P, NB, R], f16)
            nc.sync.dma_start(out=B_sb[:], in_=Bt[:])
            # x block 0 in quarters so mm1 starts on the first ~512 KB
            nq0 = 4 if NB % 4 == 0 else 2
            w0 = NB // nq0
            for q in range(nq0):
                nc.sync.dma_start(
                    out=xT_sbs[0][:, q * w0 : (q + 1) * w0, :],
                    in_=xT[0, :, q * w0 : (q + 1) * w0, :],
                )
            # A loaded compact 4x straight into mm2's row-group positions,
            # on the (otherwise idle at this point) gpsimd SWDGE ring so it
            # doesn't delay x loads; gates mm2 only.
            A_sb = const_pool.tile([P, OUT], f16)
            for j in range(4):
                nc.gpsimd.dma_start(out=A_sb[32 * j : 32 * j + R, :], in_=Af[:])
            # remaining x blocks: single fat DMAs, all dispatched up-front
            # (bufs=ntb -> no buffer-reuse waits, queue always deep)
            for tbi in range(1, ntb):
                nc.sync.dma_start(out=xT_sbs[tbi][:], in_=xT[tbi])

            # Software-pipelined by one block: PE order is
            # mm1(b), mm1(b+1), mm2(b), mm1(b+2), mm2(b+1) ... so the DVE
            # fold of block b runs under mm1(b+1) instead of stalling the PE
            # between mm1(b) and mm2(b).
            def emit_front(tbi):
                xT_sb = xT_sbs[tbi]
                # mm1, 4-way column-group packed
                ps_part = ps1.tile([P, tb], f32)
                for c8 in range(NB // 4):
                    for g in range(4):
                        c = c8 * 4 + g
                        nc.tensor.matmul(
                            ps_part[32 * g : 32 * g + R, :],
                            lhsT=B_sb[:, c, :],
                            rhs=xT_sb[:, c, :],
                            start=(c8 == 0),
                            stop=(c8 == NB // 4 - 1),
                            tile_position=(0, 32 * g),
                            skip_group_check=True,
                        )
                # fold the 4 col-group partials with a DVE chain (only one
                # PSUM operand is legal per op), landing straight in mm2's
                # row-group weight layout
                xbt_sb = xbt_pool.tile([P, P], f16, name=f"xbt{tbi}", tag="xbt")
                pa = xbt_pool.tile([R, tb], f32, name=f"pa{tbi}", tag="pa")
                pb = xbt_pool.tile([R, tb], f32, name=f"pb{tbi}", tag="pb")
                nc.vector.tensor_copy(pa[:], ps_part[0:R, :])
                nc.vector.tensor_add(pa[:], ps_part[32 : 32 + R, :], pa[:])
                nc.vector.tensor_add(pb[:], ps_part[64 : 64 + R, :], pa[:])
                for j in range(nst):
                    nc.vector.tensor_add(
                        xbt_sb[32 * j : 32 * j + R, :],
                        ps_part[96 : 96 + R, j * P : (j + 1) * P],
                        pb[:, j * P : (j + 1) * P],
                    )
                return xbt_sb

            def emit_back(tbi, xbt_sb):
                # subtile-outer: each subtile's 8 output chunks complete
                # back-to-back so its 1 MB store dispatches early
                for st in range(nst):
                    o_sb = out_pool.tile(
                        [P, OUT], f16, name=f"osb{st}_{tbi}", tag=f"osb{st}"
                    )
                    for o in range(OUT // 512):
                        ps_o = ps2.tile([P, 512], f32, tag="ps2")
                        nc.tensor.matmul(
                            ps_o[:],
                            lhsT=xbt_sb[32 * st : 32 * st + R, :],
                            rhs=A_sb[32 * st : 32 * st + R, o * 512 : (o + 1) * 512],
                            start=True,
                            stop=True,
                            tile_position=(32 * st, 0),
                            skip_group_check=True,
                        )
                        dst = o_sb[:, o * 512 : (o + 1) * 512]
                        if st % 2 == 0:
                            nc.vector.tensor_copy(dst, ps_o[:])
                        else:
                            nc.scalar.activation(
                                dst, ps_o[:], mybir.ActivationFunctionType.Copy
                            )
                    t0 = tbi * tb + st * P
                    if st % 2 == 0:
                        # DVE-copied subtile: store from idle gpsimd (SWDGE)
                        nc.gpsimd.dma_start(out=out[t0 : t0 + P, :], in_=o_sb[:])
                    else:
                        # ACT-copied subtile: store from ACT itself — the
                        # dispatch's wait is on ACT's own previous copy, so
                        # it never stalls the engine
                        nc.scalar.dma_start(out=out[t0 : t0 + P, :], in_=o_sb[:])

            prev = None
            for tbi in range(ntb):
                xbt = emit_front(tbi)
                if prev is not None:
                    emit_back(prev[0], prev[1])
                prev = (tbi, xbt)
            emit_back(prev[0], prev[1])

    nc.compile()
    _NC_CACHE[key] = nc
    return nc


TB = 256


def make_in_maps(x, lora_A, lora_B, n_cores=N_CORES):
    x = np.asarray(x, dtype=np.float32)
    A = np.asarray(lora_A, dtype=np.float32)
    B = np.asarray(lora_B, dtype=np.float32)
    xf = x.reshape(-1, IN)
    ntok = xf.shape[0] // n_cores
    tb = min(TB, ntok)
    A_scaled = (A * np.float32(SCALE)).astype(np.float16)
    B_resh = np.ascontiguousarray(
        B.reshape(NB, P, R).transpose(1, 0, 2), dtype=np.float16
    )
    in_maps = []
    for c in range(n_cores):
        shard = xf[c * ntok : (c + 1) * ntok]
        # pre-tile partition-major: [ntb, 128, NB, tb];
        # xt[tbi, p, c, t] = shard[tbi*tb + t, c*128 + p]
        xt = np.ascontiguousarray(
            shard.reshape(ntok // tb, tb, NB, P).transpose(0, 3, 2, 1),
            dtype=np.float16,
        )
        in_maps.append(
            {
                "xT": xt,
                "Bt": B_resh,
                "Af": A_scaled,
            }
        )
    return in_maps, ntok


def kernel_with_results(x, lora_A, lora_B, trace=False, **kwargs):
    from concourse.bass_utils import run_bass_kernel_spmd

    in_maps, ntok = make_in_maps(x, lora_A, lora_B)
    nc = build_nc(ntok, tb=TB)
    res = run_bass_kernel_spmd(nc, in_maps, list(range(N_CORES)), trace=trace, **kwargs)
    out = np.concatenate([r["out"] for r in res.results], axis=0).astype(np.float32)
    return out.reshape(np.asarray(x).shape[:-1] + (OUT,)), res


def kernel(x, lora_A, lora_B):
    out, _ = kernel_with_results(x, lora_A, lora_B)
    return out


# revision 3
# speedup vs baseline: 1.1558x; 1.1558x over previous
"""LoRA layer kernel for Trainium2 (Bass/Tile), data-parallel over 8 NeuronCores.

Math:  out = (x @ B) @ A * (32/16)   with x [4,2048,4096], B [4096,16], A [16,4096].

Design (HBM-bound: ~8 MB in + ~8 MB out per core at f16; floor ~42-47 us):
  - Flatten tokens (4*2048=8192), shard 1024 tokens per core (data parallel).
  - x pre-tiled PARTITION-MAJOR on host as [ntb, 128, NB, tb] f16. Loads use
    >=8 KB-per-partition descriptors ONLY (small descriptors measured ~2x
    slower per byte on HW): x block 0 in halves (8 KB), later blocks whole
    (16 KB). All loads dispatched up-front (x fully SBUF-buffered, bufs=ntb).
  - Two HWDGE rings so load and store streams interleave at SDMA packet
    granularity instead of serializing in one FIFO:
      q1  (nc.sync):   B, x loads, even-subtile stores
      q10 (nc.scalar): A112, odd-subtile stores
  - mm1: 4-way column-group packed fp16 matmuls with B zero-padded to 32
    cols per group; chunk 4k+g accumulates into PSUM rows [32g, 32g+32)
    where rows 32g+16.. are exact zeros (from the zero padding).
  - NO DVE fold: ONE [128, tb] PSUM->SBUF f16 copy moves all 4 partial
    groups (zeros included) into xbp; mm2 contracts K=128 against
    A112 [128, 4096] = A*scale at rows 32g..32g+16, ZEROS elsewhere
    (host-prepped; zeros must be real to avoid 0*garbage=NaN).
    The zero rows annihilate the padding => implicit fold, full PE rows.
  - mm2: per token-subtile st: lhsT = xbp[:, st*128:(st+1)*128] loaded once,
    8 N=512 MULTs stream A112. Output PSUM tiles are [128, 1024] (2 banks,
    2 matmuls each) so PSUM->SBUF copies are [128, 1024] — half the
    per-instruction overhead. Copies: even subtile -> DVE, odd -> ACT.
  - Stores [128, 4096] (8 KB descriptors): odd subtiles from ACT itself
    (dispatch self-gated on ACT's own copies — never stalls), even subtiles
    from sync (idle after load dispatch).
  - 8 PE warm-up matmuls against B right after the B load keep the PE busy
    ~2 us before x arrives so it leaves the low DVFS p-state early.
"""

import os
import numpy as np

IN = 4096
OUT = 4096
R = 16
N_CORES = 8
SCALE = 32.0 / 16.0
P = 128
NB = IN // P  # 32 contraction chunks


def _install_profile_hook():
    """Best-effort: register the axon NTFF profiling hook that this image's
    `antenv` package is missing, so run_bass_kernel_spmd(trace=True) can
    return exec_time_ns. Harmless no-op when anything is unavailable."""
    try:
        import sys
        import types

        if "antenv.axon_hooks" in sys.modules:
            return
        try:
            import antenv  # noqa: F401
        except ImportError:
            return
        mod = types.ModuleType("antenv.axon_hooks")
        mod._hook = None

        def set_axon_ntff_profile_hook(h):
            mod._hook = h

        def get_axon_ntff_profile_hook():
            return mod._hook

        mod.set_axon_ntff_profile_hook = set_axon_ntff_profile_hook
        mod.get_axon_ntff_profile_hook = get_axon_ntff_profile_hook
        sys.modules["antenv.axon_hooks"] = mod
        import antenv as _antenv

        _antenv.axon_hooks = mod

        so_path = "/opt/axon/libaxon_pjrt.so"
        if os.path.exists(so_path):
            try:
                from trn_agent_boot.trn_boot import _ntff_profile_via_ctypes

                hook = _ntff_profile_via_ctypes(so_path)
                if hook is not None:
                    mod._hook = hook
            except Exception:
                pass
    except Exception:
        pass


_install_profile_hook()

_NC_CACHE = {}


def build_nc(tok, tb=256):
    """Build + compile the per-core Bass program for `tok` tokens/core."""
    key = (tok, tb)
    if key in _NC_CACHE:
        return _NC_CACHE[key]

    import concourse.bacc as bacc
    import concourse.tile as tile
    from concourse import mybir

    f32 = mybir.dt.float32
    f16 = mybir.dt.float16
    tb = min(tb, tok)
    assert tok % tb == 0 and tb % P == 0
    ntb = tok // tb
    nst = tb // P  # token subtiles per block

    nc = bacc.Bacc("TRN2", target_bir_lowering=False, debug=False)
    xT = nc.dram_tensor("xT", [ntb, P, NB, tb], f16, kind="ExternalInput").ap()
    Bt = nc.dram_tensor("Bt", [P, NB, 2 * R], f16, kind="ExternalInput").ap()
    A112 = nc.dram_tensor("A112", [P, OUT], f16, kind="ExternalInput").ap()
    out = nc.dram_tensor("out", [tok, OUT], f16, kind="ExternalOutput").ap()

    with tile.TileContext(nc) as tc:
        with (
            tc.tile_pool(name="const", bufs=1) as const_pool,
            tc.tile_pool(name="xin", bufs=ntb) as x_pool,
            tc.tile_pool(name="xbp", bufs=2) as xbp_pool,
            tc.tile_pool(name="ps1", bufs=2, space="PSUM") as ps1,
            tc.tile_pool(name="ps2", bufs=3, space="PSUM") as ps2,
            tc.tile_pool(name="osb", bufs=min(4, ntb)) as out_pool,
        ):
            xT_sbs = [
                x_pool.tile([P, NB, tb], f16, name=f"x{i}", tag="x")
                for i in range(ntb)
            ]
            # tiny B first: it gates every mm1 LDWEIGHTS and the PE warm-up
            B_sb = const_pool.tile([P, NB, 2 * R], f16)
            nc.sync.dma_start(out=B_sb[:], in_=Bt[:])
            # x block 0 in halves (8 KB descriptors) so mm1 starts early
            nh = NB // 2
            for q in range(2):
                nc.sync.dma_start(
                    out=xT_sbs[0][:, q * nh : (q + 1) * nh, :],
                    in_=xT[0, :, q * nh : (q + 1) * nh, :],
                )
            # A (with zero rows) on the scalar HWDGE ring, parallel with x0;
            # gates mm2 only.
            A_sb = const_pool.tile([P, OUT], f16)
            nc.scalar.dma_start(out=A_sb[:], in_=A112[:])
            # remaining x blocks: single fat DMAs (16 KB descriptors), all
            # dispatched up-front (bufs=ntb -> no buffer-reuse waits)
            for tbi in range(1, ntb):
                nc.sync.dma_start(out=xT_sbs[tbi][:], in_=xT[tbi])

            # PE warm-up: 8 dependency-free matmuls on B data right after
            # the B load; keeps the PE clock ramping while x0 streams in.
            warm_ps = ps1.tile([P, tb], f32, name="warm", tag="ps1")
            for w in range(8):
                nc.tensor.matmul(
                    warm_ps[0 : 2 * R, :],
                    lhsT=B_sb[:, 0, :],
                    rhs=B_sb[:, (w % 2) * (NB // 4) : (w % 2) * (NB // 4) + tb // (2 * R), :],
                    start=True,
                    stop=True,
                    tile_position=(0, 0),
                    skip_group_check=True,
                )

            # Software-pipelined by one block: PE order is
            # mm1(b), mm1(b+1), mm2(b), mm1(b+2), mm2(b+1) ...
            def emit_front(tbi):
                xT_sb = xT_sbs[tbi]
                # mm1, 4-way column-group packed; group g writes rows
                # [32g, 32g+32) with the top 16 rows exact zeros
                ps_part = ps1.tile([P, tb], f32, name=f"ps1_{tbi}", tag="ps1")
                for c8 in range(NB // 4):
                    for g in range(4):
                        c = c8 * 4 + g
                        nc.tensor.matmul(
                            ps_part[32 * g : 32 * g + 2 * R, :],
                            lhsT=B_sb[:, c, :],
                            rhs=xT_sb[:, c, :],
                            start=(c8 == 0),
                            stop=(c8 == NB // 4 - 1),
                            tile_position=(0, 32 * g),
                            skip_group_check=True,
                        )
                # single full-width copy of all 4 partial groups (+ zeros)
                # into mm2's K=128 weight layout; alternate engine per block
                xbp_sb = xbp_pool.tile([P, tb], f16, name=f"xbp{tbi}", tag="xbp")
                if tbi % 2 == 0:
                    nc.vector.tensor_copy(xbp_sb[:], ps_part[:])
                else:
                    nc.scalar.activation(
                        xbp_sb[:], ps_part[:], mybir.ActivationFunctionType.Copy
                    )
                return xbp_sb

            def emit_back(tbi, xbp_sb):
                # subtile-outer: each subtile's output completes back-to-back
                # so its 1 MB store dispatches early
                for st in range(nst):
                    o_sb = out_pool.tile(
                        [P, OUT], f16, name=f"osb{st}_{tbi}", tag=f"osb{st}"
                    )
                    for o2 in range(OUT // 1024):
                        ps_o = ps2.tile([P, 1024], f32, tag="ps2")
                        for q in range(2):
                            nc.tensor.matmul(
                                ps_o[:, q * 512 : (q + 1) * 512],
                                lhsT=xbp_sb[:, st * P : (st + 1) * P],
                                rhs=A_sb[:, o2 * 1024 + q * 512 : o2 * 1024 + (q + 1) * 512],
                                start=True,
                                stop=True,
                                tile_position=(0, 0),
                                skip_group_check=True,
                            )
                        dst = o_sb[:, o2 * 1024 : (o2 + 1) * 1024]
                        if st % 2 == 0:
                            nc.vector.tensor_copy(dst, ps_o[:])
                        else:
                            nc.scalar.activation(
                                dst, ps_o[:], mybir.ActivationFunctionType.Copy
                            )
                    t0 = tbi * tb + st * P
                    if st % 2 == 0:
                        # DVE-copied subtile: store from sync (idle after the
                        # up-front load dispatches)
                        nc.sync.dma_start(out=out[t0 : t0 + P, :], in_=o_sb[:])
                    else:
                        # ACT-copied subtile: store from ACT itself — the
                        # dispatch's wait is ACT's own previous copy, so it
                        # never stalls the engine
                        nc.scalar.dma_start(out=out[t0 : t0 + P, :], in_=o_sb[:])

            prev = None
            for tbi in range(ntb):
                xbp = emit_front(tbi)
                if prev is not None:
                    emit_back(prev[0], prev[1])
                prev = (tbi, xbp)
            emit_back(prev[0], prev[1])

    nc.compile()
    _NC_CACHE[key] = nc
    return nc


TB = 256


def make_in_maps(x, lora_A, lora_B, n_cores=N_CORES):
    x = np.asarray(x, dtype=np.float32)
    A = np.asarray(lora_A, dtype=np.float32)
    B = np.asarray(lora_B, dtype=np.float32)
    xf = x.reshape(-1, IN)
    ntok = xf.shape[0] // n_cores
    tb = min(TB, ntok)
    A_scaled = (A * np.float32(SCALE)).astype(np.float16)
    # A112: rows 32g..32g+16 hold A*scale, everything else EXACT zeros
    # (multiplied against the zero-padded partial rows => implicit fold)
    A112 = np.zeros((P, OUT), dtype=np.float16)
    for g in range(4):
        A112[32 * g : 32 * g + R] = A_scaled
    B_resh = np.zeros((P, NB, 2 * R), dtype=np.float16)
    B_resh[:, :, :R] = B.reshape(NB, P, R).transpose(1, 0, 2)
    in_maps = []
    for c in range(n_cores):
        shard = xf[c * ntok : (c + 1) * ntok]
        # pre-tile partition-major: [ntb, 128, NB, tb];
        # xt[tbi, p, c, t] = shard[tbi*tb + t, c*128 + p]
        xt = np.ascontiguousarray(
            shard.reshape(ntok // tb, tb, NB, P).transpose(0, 3, 2, 1),
            dtype=np.float16,
        )
        in_maps.append(
            {
                "xT": xt,
                "Bt": B_resh,
                "A112": A112,
            }
        )
    return in_maps, ntok


def kernel_with_results(x, lora_A, lora_B, trace=False, **kwargs):
    from concourse.bass_utils import run_bass_kernel_spmd

    in_maps, ntok = make_in_maps(x, lora_A, lora_B)
    nc = build_nc(ntok, tb=TB)
    res = run_bass_kernel_spmd(nc, in_maps, list(range(N_CORES)), trace=trace, **kwargs)
    out = np.concatenate([r["out"] for r in res.results], axis=0).astype(np.float32)
    return out.reshape(np.asarray(x).shape[:-1] + (OUT,)), res


def kernel(x, lora_A, lora_B):
    out, _ = kernel_with_results(x, lora_A, lora_B)
    return out
